# revision 7
# baseline (speedup 1.0000x reference)
"""Trainium2 Bass kernel for nn_DecoderAutoreg (FaceFormer-style autoregressive
transformer decoder).

Reformulation (validated to ~1.7e-6 absmax vs the reference on CPU):
- The reference re-runs the full decoder on a growing sequence every iteration;
  causal masking makes position outputs length-invariant, so the computation is
  exactly a KV-cache decode: 32 sequential single-token steps.
- The cross-attention memory mask is diagonal, so its softmax is exactly
  one-hot in fp32: cross-attention collapses to a per-position linear term,
  precomputed on-device in a prologue.

Device layouts:
- Activations x: (128, 8) tiles, col = 2*c + b (c = D-chunk of 4x128, b = batch).
- q/k/v head-major: (64 partitions = head dim, col = 2*h + b); caches are
  (64, H, 2T), col pair 2*t+b appended per step.
- Weights-stationary matmuls, bf16 (FWL). LayerNorm stats in fp32 via
  per-partition accum_out sums + one fp32 (1/512)-matrix matmul that both
  finishes the cross-partition reduction and broadcasts stats to all partitions.
- Batched attention: scores for both batch rows in one matmul with additive
  -1e9 masking of cross-batch columns; exp() zeroes them; a ones-matmul then
  broadcasts each batch's row across 64 partitions for the DVE attn*V multiply.
"""

import numpy as np
import ml_dtypes

D, H, DFF, NL, MOTION = 512, 8, 2048, 2, 64
B, T = 2, 32
PERIOD, MAXLEN = 30, 600
EPS = 1e-5
NEG = -1e9
dh = D // H
NC_CORES = 8

_CACHE = {}


# ---------------------------------------------------------------- host tables
def _pe_table():
    pe = np.zeros((PERIOD, D), np.float32)
    pos = np.arange(PERIOD, dtype=np.float32)[:, None]
    div = np.exp(np.arange(0, D, 2, dtype=np.float32) * (-np.log(10000.0) / D))
    pe[:, 0::2] = np.sin(pos * div)
    pe[:, 1::2] = np.cos(pos * div)
    return np.tile(pe, (MAXLEN // PERIOD + 1, 1))[:T]  # (T, D)


def _alibi():
    start = 2.0 ** (-(2.0 ** -(np.log2(H) - 3)))
    slopes = np.array([start * start ** i for i in range(H)], np.float32)
    bias = (np.repeat(np.arange(0, MAXLEN, PERIOD), PERIOD) // PERIOD)
    bias = (-bias[::-1]).astype(np.float32)
    al = np.zeros((H, T, T), np.float32)
    for i in range(T):
        al[:, i, :i + 1] = slopes[:, None] * bias[MAXLEN - (i + 1):MAXLEN - (i + 1) + i + 1][None]
    return al  # (H, T, T)


def _sbias_packed():
    """Per-step score bias, packed: step-t block width 16(t+1) at offset
    16*t(t+1)/2; [b, h*2(t+1) + 2j + b2] = AL[h,t,j] + NEG*(b != b2)."""
    al = _alibi()
    tab = np.zeros((2, 16 * (T * (T + 1) // 2)), np.float32)
    off = 0
    for t in range(T):
        w = 2 * (t + 1)
        for b in range(2):
            blk = np.zeros((H, w), np.float32)
            for b2 in range(2):
                blk[:, b2::2] = al[:, t, :t + 1] + (NEG if b2 != b else 0.0)
            tab[b, off:off + H * w] = blk.reshape(-1)
        off += H * w
    return tab


def _sbias_off(t):
    return 16 * (t * (t + 1) // 2)


# ---------------------------------------------------------------- host prep
def _rep128(vec, w):
    """(n*128,) -> (128, w) with col 2c+b = vec[c*128+p], replicated over b."""
    vec = np.asarray(vec, np.float32)
    t = np.zeros((128, w), np.float32)
    for c in range(w // 2):
        for b in range(2):
            t[:, 2 * c + b] = vec[c * 128:(c + 1) * 128]
    return t


def _prep_inputs(I):
    bf = ml_dtypes.bfloat16
    f32 = np.float32
    P = {}

    def cvt(a, dt=bf):
        return np.ascontiguousarray(np.asarray(a, f32).astype(dt))

    for l in range(NL):
        P[f"saqkvT{l}"] = cvt(I["sa_in_w"][l].T)             # (512, 1536)
        P[f"saoutT{l}"] = cvt(I["sa_out_w"][l].T)            # (512, 512)
        P[f"ff1T{l}"] = cvt(I["ff_w1"][l].T)                 # (512, 2048)
        P[f"ff2T{l}"] = cvt(I["ff_w2"][l].T)                 # (2048, 512)
        P[f"cavT{l}"] = cvt(np.asarray(I["ca_in_w"][l], f32)[2 * D:3 * D].T)
        P[f"caoutT{l}"] = cvt(I["ca_out_w"][l].T)            # (512, 512)

        sab = np.asarray(I["sa_in_b"][l], f32)
        for nm, base, scale in (("bqs", 0, 0.125), ("bks", D, 1.0), ("bvs", 2 * D, 1.0)):
            t = np.zeros((64, 16), f32)
            for h in range(H):
                for b in range(2):
                    t[:, 2 * h + b] = sab[base + h * dh:base + (h + 1) * dh] * scale
            P[f"{nm}{l}"] = t
        P[f"bo{l}"] = _rep128(I["sa_out_b"][l], 8)
        P[f"b1f{l}"] = _rep128(I["ff_b1"][l], 32)
        P[f"b2f{l}"] = _rep128(I["ff_b2"][l], 8)
        P[f"bvc{l}"] = _rep128(np.asarray(I["ca_in_b"][l], f32)[2 * D:3 * D], 8)
        P[f"boc{l}"] = _rep128(I["ca_out_b"][l], 8)
        for j in range(3):
            P[f"g{l}{j}"] = _rep128(np.asarray(I["ln_g"], f32)[l, j], 8)
            P[f"bt{l}{j}"] = _rep128(np.asarray(I["ln_b"], f32)[l, j], 8)

    P["mmrT"] = cvt(I["mmr_w"].T)                            # (512, 64)
    P["mmT"] = cvt(I["mm_w"].T)                              # (64, 512)
    P["initT"] = cvt(np.asarray(I["init_state"], f32).T)     # (64, 2)
    P["contentT"] = cvt(np.asarray(I["content_code"], f32).transpose(2, 1, 0).reshape(D, T * B))
    P["mmrb"] = np.asarray(I["mmr_b"], f32).reshape(64, 1).copy()
    P["mmb_s"] = _rep128(I["mm_b"], 8)

    sty = np.asarray(I["style_code"], f32)
    t = np.zeros((128, 8), np.float32)
    for c in range(4):
        for b in range(2):
            t[:, 2 * c + b] = sty[b, c * 128:(c + 1) * 128]
    P["style_s"] = t

    pe = _pe_table()
    t = np.zeros((128, 8 * T), np.float32)
    for tau in range(T):
        for c in range(4):
            for b in range(2):
                t[:, 8 * tau + 2 * c + b] = pe[tau, c * 128:(c + 1) * 128]
    P["PEc"] = t
    P["sbias"] = _sbias_packed()
    P["recip512"] = np.full((128, 128), 1.0 / 512.0, np.float32)
    P["ones2"] = np.ones((2, 64), ml_dtypes.bfloat16)
    P["epsc"] = np.full((128, 1), EPS, np.float32)
    return P


# ---------------------------------------------------------------- wait split
def _split_sync_waits(nc, mybir, maxw=1):
    """walrus CoreV3 codegen rejects instructions carrying more than one sync
    wait; hoist excess waits onto same-engine NoOps inserted just before."""
    n_new = 0
    for f in nc.m.functions:
        for blk in f.blocks:
            new_insts = []
            for ins in blk.instructions:
                si = ins.sync_info
                if si is not None and si.on_wait and len(si.on_wait) > maxw:
                    waits = list(si.on_wait)
                    head, rest = waits[:-maxw], waits[-maxw:]
                    for ci in range(0, len(head), maxw):
                        nop = mybir.InstNoOp(name=f"I-ws-{n_new}", ins=[], outs=[])
                        n_new += 1
                        nop.engine = ins.engine
                        nop.sync_info = mybir.SyncInfo(on_wait=head[ci:ci + maxw], on_update=[])
                        new_insts.append(nop)
                    ins.sync_info = mybir.SyncInfo(on_wait=rest, on_update=list(si.on_update))
                new_insts.append(ins)
            if len(new_insts) != len(blk.instructions):
                blk.instructions[:] = new_insts
    return n_new


# ---------------------------------------------------------------- bass build
def _build(P):
    import contextlib
    import concourse.bass as bass
    import concourse.tile as tile
    from concourse import mybir

    f32 = mybir.dt.float32
    bf16 = mybir.dt.bfloat16
    OP = mybir.AluOpType
    ACT = mybir.ActivationFunctionType
    X = mybir.AxisListType.X

    nc = bass.Bass()

    prm = {}
    for k, v in P.items():
        dt = bf16 if v.dtype == ml_dtypes.bfloat16 else f32
        prm[k] = nc.declare_dram_parameter(k, list(v.shape), dt, isOutput=False)
    out_d = nc.declare_dram_parameter("out", [64, 64], f32, isOutput=True)

    with tile.TileContext(nc) as tc:
        ctx = contextlib.ExitStack()
        state = ctx.enter_context(tc.tile_pool(name="state", bufs=1))
        work = ctx.enter_context(tc.tile_pool(name="work", bufs=2))
        ps = ctx.enter_context(tc.tile_pool(name="ps", bufs=1, space="PSUM"))
        ps2 = ctx.enter_context(tc.tile_pool(name="ps2", bufs=2, space="PSUM"))

        sb = {}

        def load(name, kparts, cols, dt, ksplit=128):
            if kparts > ksplit:
                nkc = kparts // ksplit
                tl = state.tile([ksplit, nkc, cols], dt, tag=name)
                for kc in range(nkc):
                    nc.sync.dma_start(out=tl[:, kc, :],
                                      in_=prm[name][kc * ksplit:(kc + 1) * ksplit, :])
            else:
                tl = state.tile([kparts, cols], dt, tag=name)
                nc.sync.dma_start(out=tl[:, :], in_=prm[name][:, :])
            sb[name] = tl

        for l in range(NL):
            load(f"saqkvT{l}", 512, 1536, bf16)
            load(f"ff1T{l}", 512, 2048, bf16)
            load(f"ff2T{l}", 2048, 512, bf16)
            load(f"cavT{l}", 512, 512, bf16)
            load(f"caoutT{l}", 512, 512, bf16)
            load(f"saoutT{l}", 512, 512, bf16, ksplit=64)   # K=64 chunks
            for nm in ("bqs", "bks", "bvs"):
                load(f"{nm}{l}", 64, 16, f32)
            for nm, w in (("bo", 8), ("b1f", 32), ("b2f", 8), ("bvc", 8), ("boc", 8)):
                load(f"{nm}{l}", 128, w, f32)
            for j in range(3):
                load(f"g{l}{j}", 128, 8, f32)
                load(f"bt{l}{j}", 128, 8, f32)
        load("mmrT", 512, 64, bf16)
        load("mmT", 64, 512, bf16)
        load("initT", 64, 2, bf16)
        load("contentT", 512, 64, bf16)
        load("mmrb", 64, 1, f32)
        load("mmb_s", 128, 8, f32)
        load("style_s", 128, 8, f32)
        load("PEc", 128, 8 * T, f32)
        load("sbias", 2, 16 * (T * (T + 1) // 2), f32)
        load("recip512", 128, 128, f32)
        load("ones2", 2, 64, bf16)
        load("epsc", 128, 1, f32)

        # persistent state
        kcache = [state.tile([64, H, 2 * T], bf16, tag=f"kc{l}", name=f"kc{l}") for l in range(NL)]
        vcache = [state.tile([64, H, 2 * T], f32, tag=f"vc{l}", name=f"vc{l}") for l in range(NL)]
        xfin_bf = state.tile([128, 8 * T], bf16, tag="xfin")
        x_bf = state.tile([128, 8], bf16, tag="x_bf")
        x_f = state.tile([128, 8], f32, tag="x_f")
        add3 = state.tile([128, 8 * T], f32, tag="add3")
        ca_add = [state.tile([128, 8 * T], f32, tag=f"ca{l}", name=f"ca{l}") for l in range(NL)]

        def cb(ap):  # (128, 8) -> (128, 4, 2)
            return ap.rearrange("p (c b) -> p c b", b=2)

        # ================= prologue =================
        a3v = add3.rearrange("p (t c) -> p t c", c=8)
        nc.vector.tensor_add(out=a3v, in0=sb["PEc"].rearrange("p (t c) -> p t c", c=8),
                             in1=sb["mmb_s"][:, None, :].broadcast_to([128, T, 8]))
        nc.vector.tensor_add(out=a3v, in0=a3v,
                             in1=sb["style_s"][:, None, :].broadcast_to([128, T, 8]))

        for l in range(NL):
            vct = work.tile([128, 4, 64], bf16, tag="vct")
            for mc in range(4):
                pv = ps.tile([128, 64], f32, tag="pbc")
                for kc in range(4):
                    nc.tensor.matmul(pv[:, :], sb[f"cavT{l}"][:, kc, mc * 128:(mc + 1) * 128],
                                     sb["contentT"][:, kc, :], start=(kc == 0), stop=(kc == 3))
                nc.vector.tensor_add(
                    out=vct[:, mc, :].rearrange("p (t b) -> p t b", b=2),
                    in0=pv.rearrange("p (t b) -> p t b", b=2),
                    in1=sb[f"bvc{l}"][:, None, 2 * mc:2 * mc + 2].broadcast_to([128, T, 2]))
            cav = ca_add[l].rearrange("p (t c b) -> p c t b", c=4, b=2)
            for mc in range(4):
                pc = ps.tile([128, 64], f32, tag="pbc")
                for kc in range(4):
                    nc.tensor.matmul(pc[:, :], sb[f"caoutT{l}"][:, kc, mc * 128:(mc + 1) * 128],
                                     vct[:, kc, :], start=(kc == 0), stop=(kc == 3))
                nc.vector.tensor_add(
                    out=cav[:, mc, :, :],
                    in0=pc.rearrange("p (t b) -> p t b", b=2),
                    in1=sb[f"boc{l}"][:, None, 2 * mc:2 * mc + 2].broadcast_to([128, T, 2]))

        px = ps2.tile([128, 8], f32, tag="acc8")
        for mc in range(4):
            nc.tensor.matmul(px[:, 2 * mc:2 * mc + 2], sb["mmT"][:, mc * 128:(mc + 1) * 128],
                             sb["initT"][:, :], start=True, stop=True)
        nc.vector.tensor_add(out=x_f[:, :], in0=px[:, :], in1=add3[:, 0:8])
        nc.vector.tensor_copy(out=x_bf[:, :], in_=x_f[:, :])

        # ---------------- LayerNorm ----------------
        def layer_norm(u, pps, l, j, out_ap, also_bf=None):
            scr = work.tile([128, 8], f32, tag="lnscr")
            for b in range(2):
                nc.vector.scalar_tensor_tensor(
                    out=cb(scr)[:, :, b], in0=cb(u)[:, :, b], scalar=1.0,
                    in1=cb(u)[:, :, b], op0=OP.mult, op1=OP.mult,
                    accum_out=pps[:, 2 + b:3 + b])
            mst = ps.tile([128, 4], f32, tag="mst")
            nc.tensor.matmul(mst[:, :], sb["recip512"][:, :], pps[:, :], start=True, stop=True)
            m2 = work.tile([128, 4], f32, tag="lnm2")
            nc.scalar.activation(out=m2[:, 0:2], in_=mst[:, 0:2],
                                 func=ACT.Square)
            nc.vector.tensor_sub(out=m2[:, 2:4], in0=mst[:, 2:4], in1=m2[:, 0:2])
            sd = work.tile([128, 2], f32, tag="lnsd")
            nc.scalar.activation(out=sd[:, :], in_=m2[:, 2:4], func=ACT.Sqrt,
                                 bias=sb["epsc"][:, :], scale=1.0)
            nc.vector.reciprocal(out=sd[:, :], in_=sd[:, :])
            t1 = work.tile([128, 8], f32, tag="lnt1")
            for b in range(2):
                nc.vector.tensor_scalar(
                    out=cb(t1)[:, :, b], in0=cb(u)[:, :, b],
                    scalar1=mst[:, b:b + 1], scalar2=sd[:, b:b + 1],
                    op0=OP.subtract, op1=OP.mult)
            nc.vector.tensor_mul(out=t1[:, :], in0=t1[:, :], in1=sb[f"g{l}{j}"][:, :])
            # out_ap must be a (128, 4, 2) view
            nc.vector.tensor_add(out=out_ap, in0=cb(t1), in1=cb(sb[f"bt{l}{j}"]))
            if also_bf is not None:
                nc.vector.tensor_copy(out=also_bf, in_=out_ap)

        # ================= 32 decode steps =================
        for t in range(T):
            np1 = t + 1
            width = H * 2 * np1
            for l in range(NL):
                qkvT = sb[f"saqkvT{l}"]
                pqkv = ps.tile([64, 48], f32, tag="qkv")
                for part, base in ((0, 0), (16, 512), (32, 1024)):
                    for h in range(H):
                        for kc in range(4):
                            nc.tensor.matmul(
                                pqkv[:, part + 2 * h:part + 2 * h + 2],
                                qkvT[:, kc, base + h * 64:base + (h + 1) * 64],
                                x_bf[:, 2 * kc:2 * kc + 2],
                                start=(kc == 0), stop=(kc == 3))
                q_sb = work.tile([64, 16], bf16, tag="q_sb")
                nc.vector.scalar_tensor_tensor(out=q_sb[:, :], in0=pqkv[:, 0:16],
                                               scalar=0.125, in1=sb[f"bqs{l}"][:, :],
                                               op0=OP.mult, op1=OP.add)
                nc.vector.tensor_add(
                    out=kcache[l].rearrange("p h (t b) -> p h t b", b=2)[:, :, t, :],
                    in0=pqkv[:, 16:32].rearrange("p (h b) -> p h b", b=2),
                    in1=sb[f"bks{l}"].rearrange("p (h b) -> p h b", b=2))
                nc.vector.tensor_add(
                    out=vcache[l].rearrange("p h (t b) -> p h t b", b=2)[:, :, t, :],
                    in0=pqkv[:, 32:48].rearrange("p (h b) -> p h b", b=2),
                    in1=sb[f"bvs{l}"].rearrange("p (h b) -> p h b", b=2))

                # scores, packed head blocks of width 2*np1
                psc = ps.tile([2, 512], f32, tag="ps")
                for h in range(H):
                    nc.tensor.matmul(psc[:, h * 2 * np1:(h + 1) * 2 * np1],
                                     q_sb[:, 2 * h:2 * h + 2],
                                     kcache[l][:, h, 0:2 * np1], start=True, stop=True)
                s2 = work.tile([2, 512], f32, tag="s2")
                nc.vector.tensor_add(out=s2[:, 0:width], in0=psc[:, 0:width],
                                     in1=sb["sbias"][:, _sbias_off(t):_sbias_off(t) + width])
                e_sb = work.tile([2, 512], bf16, tag="e_sb")
                nc.scalar.activation(out=e_sb[:, 0:width], in_=s2[:, 0:width], func=ACT.Exp)
                pbc = ps.tile([64, 512], f32, tag="pbc")
                nc.tensor.matmul(pbc[:, 0:width], sb["ones2"][:, :], e_sb[:, 0:width],
                                 start=True, stop=True)
                pbcv = pbc[:, 0:width].rearrange("p (h t b) -> p h b t", b=2, t=np1)
                vcv = vcache[l].rearrange("p h (t b) -> p h b t", b=2)[:, :, :, 0:np1]
                tmp = work.tile([64, H, 2, T], f32, tag="avtmp")
                nc.vector.tensor_mul(out=tmp[:, :, :, 0:np1], in0=vcv, in1=pbcv)
                o2 = work.tile([64, 16], f32, tag="o2")
                nc.vector.tensor_reduce(out=o2.rearrange("p (h b) -> p h b", b=2),
                                        in_=tmp[:, :, :, 0:np1], axis=X, op=OP.add)
                sums = work.tile([64, 16], f32, tag="sums")
                nc.vector.tensor_reduce(out=sums.rearrange("p (h b) -> p h b", b=2),
                                        in_=pbcv, axis=X, op=OP.add)
                nc.vector.reciprocal(out=sums[:, :], in_=sums[:, :])
                o_bf = work.tile([64, 16], bf16, tag="o_bf")
                nc.vector.tensor_mul(out=o_bf[:, :], in0=o2[:, :], in1=sums[:, :])

                # out-proj (K=64 chunks) + residual + LN1
                py = ps2.tile([128, 8], f32, tag="acc8")
                for mc in range(4):
                    for kc in range(8):
                        nc.tensor.matmul(py[:, 2 * mc:2 * mc + 2],
                                         sb[f"saoutT{l}"][:, kc, mc * 128:(mc + 1) * 128],
                                         o_bf[:, 2 * kc:2 * kc + 2],
                                         start=(kc == 0), stop=(kc == 7))
                u1 = work.tile([128, 8], f32, tag="u1")
                pps1 = work.tile([128, 4], f32, tag="pps1")
                nc.vector.tensor_add(out=u1[:, :], in0=py[:, :], in1=sb[f"bo{l}"][:, :])
                for b in range(2):
                    nc.vector.scalar_tensor_tensor(
                        out=cb(u1)[:, :, b], in0=cb(u1)[:, :, b], scalar=1.0,
                        in1=cb(x_f)[:, :, b], op0=OP.mult, op1=OP.add,
                        accum_out=pps1[:, b:b + 1])
                x1 = work.tile([128, 8], f32, tag="x1")
                layer_norm(u1, pps1, l, 0, cb(x1))

                # cross-attention additive term + LN2
                u2 = work.tile([128, 8], f32, tag="u2")
                pps2 = work.tile([128, 4], f32, tag="pps2")
                ca_t = ca_add[l][:, 8 * t:8 * t + 8]
                for b in range(2):
                    nc.vector.scalar_tensor_tensor(
                        out=cb(u2)[:, :, b], in0=cb(x1)[:, :, b], scalar=1.0,
                        in1=cb(ca_t)[:, :, b], op0=OP.mult, op1=OP.add,
                        accum_out=pps2[:, b:b + 1])
                x2 = work.tile([128, 8], f32, tag="x2")
                x2_bf = work.tile([128, 8], bf16, tag="x2bf")
                layer_norm(u2, pps2, l, 1, cb(x2), also_bf=cb(x2_bf))

                # feed-forward
                ph = ps.tile([128, 32], f32, tag="ph")
                for mt in range(16):
                    for kc in range(4):
                        nc.tensor.matmul(ph[:, 2 * mt:2 * mt + 2],
                                         sb[f"ff1T{l}"][:, kc, mt * 128:(mt + 1) * 128],
                                         x2_bf[:, 2 * kc:2 * kc + 2],
                                         start=(kc == 0), stop=(kc == 3))
                hb = work.tile([128, 32], f32, tag="hb")
                nc.vector.tensor_add(out=hb[:, :], in0=ph[:, :], in1=sb[f"b1f{l}"][:, :])
                h_bf = work.tile([128, 32], bf16, tag="h_bf")
                nc.vector.tensor_scalar_max(out=h_bf[:, :], in0=hb[:, :], scalar1=0.0)
                pz = ps2.tile([128, 8], f32, tag="acc8")
                for mc in range(4):
                    for kc in range(16):
                        nc.tensor.matmul(pz[:, 2 * mc:2 * mc + 2],
                                         sb[f"ff2T{l}"][:, kc, mc * 128:(mc + 1) * 128],
                                         h_bf[:, 2 * kc:2 * kc + 2],
                                         start=(kc == 0), stop=(kc == 15))
                u3 = work.tile([128, 8], f32, tag="u3")
                pps3 = work.tile([128, 4], f32, tag="pps3")
                nc.vector.tensor_add(out=u3[:, :], in0=pz[:, :], in1=sb[f"b2f{l}"][:, :])
                for b in range(2):
                    nc.vector.scalar_tensor_tensor(
                        out=cb(u3)[:, :, b], in0=cb(u3)[:, :, b], scalar=1.0,
                        in1=cb(x2)[:, :, b], op0=OP.mult, op1=OP.add,
                        accum_out=pps3[:, b:b + 1])
                if l == NL - 1:
                    # xfin is c-major: col = c*64 + 2t + b
                    xfv = xfin_bf.rearrange("p (c t b) -> p c t b", c=4, b=2)
                    layer_norm(u3, pps3, l, 2, xfv[:, :, t, :])
                else:
                    layer_norm(u3, pps3, l, 2, cb(x_f), also_bf=cb(x_bf))

            # recurrence tail
            if t < T - 1:
                pm1 = ps.tile([64, 4], f32, tag="mst")
                for kc in range(4):
                    nc.tensor.matmul(pm1[:, 0:2], sb["mmrT"][:, kc, :],
                                     xfin_bf[:, kc * 64 + 2 * t:kc * 64 + 2 * t + 2],
                                     start=(kc == 0), stop=(kc == 3))
                m1s = work.tile([64, 2], bf16, tag="m1s")
                nc.vector.tensor_scalar_add(out=m1s[:, :], in0=pm1[:, 0:2],
                                            scalar1=sb["mmrb"][:, :])
                pm2 = ps2.tile([128, 8], f32, tag="acc8")
                for mc in range(4):
                    nc.tensor.matmul(pm2[:, 2 * mc:2 * mc + 2],
                                     sb["mmT"][:, mc * 128:(mc + 1) * 128],
                                     m1s[:, :], start=True, stop=True)
                nc.vector.tensor_add(out=x_f[:, :], in0=pm2[:, :],
                                     in1=add3[:, 8 * (t + 1):8 * (t + 1) + 8])
                nc.vector.tensor_copy(out=x_bf[:, :], in_=x_f[:, :])

        # ================= epilogue =================
        po = ps.tile([64, 64], f32, tag="pbc")
        for kc in range(4):
            nc.tensor.matmul(po[:, :], sb["mmrT"][:, kc, :],
                             xfin_bf[:, kc * 64:(kc + 1) * 64], start=(kc == 0), stop=(kc == 3))
        osb = work.tile([64, 64], f32, tag="osb")
        nc.vector.tensor_scalar_add(out=osb[:, :], in0=po[:, :], scalar1=sb["mmrb"][:, :])
        nc.sync.dma_start(out=out_d[:, :], in_=osb[:, :])
        ctx.close()

    _split_sync_waits(nc, mybir, maxw=1)
    return nc


# ---------------------------------------------------------------- entry point
def kernel(**inputs):
    from concourse.bass_utils import run_bass_kernel_spmd

    P = _prep_inputs(inputs)
    if "module" not in _CACHE:
        _CACHE["module"] = _build(P)
    nc = _CACHE["module"]

    in_map = {k: np.asarray(v) for k, v in P.items()}
    res = run_bass_kernel_spmd(nc, [in_map] * NC_CORES, list(range(NC_CORES)))
    o = np.asarray(res.results[0]["out"])  # (64 motion, 64 = 2t+b)
    return np.ascontiguousarray(o.reshape(64, T, B).transpose(2, 1, 0).astype(np.float32))
